# revision 13
# baseline (speedup 1.0000x reference)
"""Trainium2 Bass kernel for the LoRA dynamics MLP.

Math: out = L2(relu(L1(relu(L0(concat(state, action))))))
with Li(x) = x @ (Wi + s*Ui@Di).T + bi  (LoRA folded into the base GEMM,
exact algebra: x@W.T + s*(x@Di.T)@Ui.T == x@(W + s*Ui@Di).T).

Distribution: pure data parallel over 8 NeuronCores (batch 65536 -> 8192
rows/core); the ~1 MB of folded weights are replicated.

Device layout: activations are feature-major ([features, batch] -- features
on SBUF partitions), so every layer is a plain
psum[mj] = sum_k WT[k, mj-slice].T @ xT[k, :] accumulation; L0/L1 bias+ReLU
are fused on the ScalarE activation (PSUM -> SBUF), L2 bias-add runs on
DVE/ScalarE. L1 matmul operands are fp16 (full PE rate, 1 cycle/row) with
fp32 PSUM accumulation.

L0 and L2 -- the two big GEMMs -- run in fp8 DoubleRow mode at 0.5 PE
cycles/row with error compensation. L0: x ships as an fp8 {hi, lo} pair
(x_hi=fp8(x), x_lo=fp8(x-x_hi), same bytes as fp16; the lo plane is
unscaled so one 64*Whi plane serves both DoubleRow halves via a broadcast
lhsT), plus cross-k {Wlo64, Wlo64} residual pairs: 11 DR instructions per
output block accumulate 64*(W0@x), and the L0 drain scales by 1/64 (fused
into the activation; the 1/64 is folded into w1 on the host so h1 is
stored 64x-scaled and the drain needs no scale operand). w0a carries 8
weight blocks: 3x{Whi64 pair}, a fused {Whi64_6, Wlo64_6} block (its v1
half rides in the dead zero plane of the k6 slot; normal tiles broadcast
only v0), 3x{Wlo64 pair}, and {Wlo64_6, 0}.

Error budget (gate: rel-L2 < 2e-2; this config measures 1.904e-2,
deterministic): tiles {0..5} skip the x_lo plane entirely -- their loads
ship half the bytes and their L0 runs a 7-instruction hi-only schedule
(3 {Whi64 pair}@{x_hi pair}, 3 {Wlo64 pair}@{x_hi pair}, and the fused
k6 block against a broadcast x_hi rhs), saving 856 ns of PE and 1274 ns
of DMA per tile at ~6.8e-3 rel-L2 each. Tiles {6..15} quarter-skip: only
the k6 group's x_lo is dropped, which is uniquely cheap (~2.5e-3 rel-L2
for 214 ns of PE each) because the fused {Whi64_6, Wlo64_6} block
absorbs k6's whole compensation in one instruction, shrinking L0 from 11
to 10 instructions. Tile 15 additionally skips the h2-residual product
in L2 (the cc plane), trimming the closing drain/store convoy. This
full+quarter allocation maximizes PE time bought per unit of squared
error (quarter-skips are ~2x more efficient than full skips).

L2: the L1 drain emits 32x-scaled fp8 h2 planes (a32=fp16(relu(32z+32b1))
on ScalarE, then b32=fp8(a32), c=fp8(a32-b32) on DVE and h4=fp8(a32/8) on
Pool), and 3 DR instructions per output block accumulate
W2hi@b32 + W2hi@c + W2lo8@h4 = 32*(W2@h2); y drains scale by 1/32 with
the bias folded in (ScalarE Identity / DVE fused mult+add).

The batch-tile loop is software-pipelined on the PE: iteration i runs
L0(i) -> L1(i-1) -> L2(i-2), so the ScalarE activation latency between
layers hides under the next tile's matmuls and the PE never stalls in
steady state (~88% busy; the kernel sits at the compute/memory ridge:
PE ~77.6 us busy, DMA ~70.4 us, ScalarE ~69.8 us over an 89.7 us span).
Startup is trimmed by interleaving slot-chunks of w0 with k-chunks of x0
(the first matmuls need only the leading chunks) and by a burst of tiny
matmuls on memset data that pins the modeled PE clock-ramp start while
the first loads are in flight. L2 drains alternate DVE/ScalarE so neither
becomes the tail bottleneck, and y stores go out in sub-tile chunks to
overlap the drain with the store DMA. Host does layout only
(transpose/concat/shard/cast/fp8-split) plus the tiny O(H*R*F) LoRA fold
in float64.

Startup choreography: the three tiny bias vectors ship as ONE merged DMA
right behind the first w0/x0 chunks -- b0 gates ScalarE's sequencer and
the one-time activation-table load, so loading it late stalls the whole
first vertical chain; x0's second chunk loads before w0's so the PE can
consume the k4..6 hi-pairs while the residual weight blocks are still in
flight.

Drain-scale folding: w1 ships as w1/2 (absorbing the 1/64 L0-drain scale
against the 64x-stored h1 AND the 32x a32 pre-scale), so both the h1 and
a32 drains are scale-free add(+bias)/max ops runnable on ScalarE or DVE
per tile (all currently on ScalarE -- every DVE placement measured worse
on the final schedule).

TimelineSim cost model: 86890 ns (baseline inherited by this session:
94162 ns).
"""

import ml_dtypes
import numpy as np

import concourse.mybir as mybir
import concourse.tile as tile
from concourse import bacc, bass_utils

P = 128
B = 65536
S = 768
A = 128
F0 = S + A            # 896
H = 256
NCORES = 8
BC = B // NCORES      # 8192 rows per core
BT = 512              # batch tile (matmul moving dim; one f32 PSUM bank)
NBT = BC // BT        # 16 batch tiles per core
KO0, KO1, KO2 = F0 // P, H // P, H // P     # 7, 2, 2 contraction tiles
MO0, MO1, MO2 = H // P, H // P, S // P      # 2, 2, 6 output tiles
LORA_SCALE = 16.0 / 8.0

F32 = mybir.dt.float32
F16 = mybir.dt.float16
F8 = mybir.dt.float8e4
F8NP = ml_dtypes.float8_e4m3  # numpy dtype matching mybir float8e4
RELU = mybir.ActivationFunctionType.Relu
IDENT = mybir.ActivationFunctionType.Identity
DR = mybir.MatmulPerfMode.DoubleRow
LO_SCALE = 64.0       # fp8 residual pre-scale; the L0 drain divides it out
NS0 = KO0 + 1 + (KO0 - 1) // 2  # 11 L0 DoubleRow instructions per mj
NB0 = 8               # w0 blocks: 3x{Whi64 pair} + {Whi64_6, Wlo64_6} + 3x{Wlo64 pair} + {Wlo64_6, 0}

_NC_CACHE = []
LAST_RESULT = None  # BassKernelResults of the most recent run (for test.py)
LAST_BOUNDS = [1, 3, 5, 6]  # y-store chunk boundaries for the final tile


def _build(xp_bufs=4, hp_bufs=3, pp_bufs=8, op_bufs=3, in_split=2, y_split=3,
           wu_n=40, wu_ap=64, tail_split=False,
           cc_pool=False, y_pattern="AAADDD", y_pattern_last=None,
           last_split=0, store_eng="sp", last_bounds=None,
           kb_list=None, sb_list=None, store_alt=(), cc_skip=(15,),
           xlo_skip=(0, 1, 2, 3, 4, 5), x_eng_alt=False, w2_after=None,
           pair_drain=False, y8_tiles=(), stage_order="012",
           pq_bufs=2, pp_bufs2=4, bias_merge=True, w2_msplit=False,
           y_pattern_14=None, tail_par=False, winddown_21=False,
           i3_special=False, w2v_split=False, ahead=3,
           h1_pattern="AAAAAAAAAAAAAAAA", cc_pool_tiles=(), wu_pool=False,
           g1_swap=True, a32_pattern="AAAAAAAAAAAAAAAA", y_pat_map=None,
           prod_order="bch", xhalf_skip=(),
           xq_skip=(6, 7, 8, 9, 10, 11, 12, 13, 14, 15), s1_pq=False):
    nc = bacc.Bacc("TRN2", target_bir_lowering=False, debug=False,
                   num_devices=NCORES)
    # L0 runs entirely in fp8 DoubleRow (0.5 PE cycles/row). x ships as an
    # fp8 hi/lo pair (same bytes as fp16): xTi[:,0]=fp8(x),
    # xTi[:,1]=fp8(x-fp8(x)) (unscaled -- the denormal floor only touches
    # negligible residuals). One 64*Whi_k plane then serves BOTH halves of
    # the main DR instructions via a broadcast lhsT
    # (64Whi@x_hi + 64Whi@x_lo), and 4 more DR instructions inject the
    # weight-residual products Wlo64_k@x_hi_k (cross-k pairs plus a
    # {Wlo64_6, zero} block), so PSUM accumulates 64*(W0@x) with both x
    # and W quantization error compensated to ~1e-3 and only 524 KB of
    # weight planes. The L0 drain scales by 1/64.
    xTi = nc.dram_tensor("xTi", [F0, 2, BC], F8, kind="ExternalInput").ap()
    w0a = nc.dram_tensor("w0a", [NB0 * P, 2 * H], F8, kind="ExternalInput").ap()
    w1t = nc.dram_tensor("w1t", [H, H], F16, kind="ExternalInput").ap()
    # L2 also runs fp8 DoubleRow: w2i = [W2hi; W2lo8] (fp8(W2) and
    # fp8(8*(W2-W2hi))), used against the 32x-scaled fp8 h2 planes built by
    # the L1 drain (see s1/s2); PSUM gets 32*(W2@h2), drains scale by 1/32.
    w2i = nc.dram_tensor("w2i", [2 * H, S], F8, kind="ExternalInput").ap()
    b0 = nc.dram_tensor("b0", [H], F32, kind="ExternalInput").ap()
    b1s = nc.dram_tensor("b1s", [H], F32, kind="ExternalInput").ap()  # 32*b1
    b2 = nc.dram_tensor("b2", [S], F32, kind="ExternalInput").ap()
    bia = (nc.dram_tensor("bia", [H + H + S], F32, kind="ExternalInput").ap()
           if bias_merge else None)
    yT = nc.dram_tensor("yT", [S, BC], F16, kind="ExternalOutput").ap()
    yT8 = (nc.dram_tensor("yT8", [S, BC], F8, kind="ExternalOutput").ap()
           if y8_tiles else None)

    w0a_t = w0a.rearrange("(s p) (v m) -> p s v m", p=P, v=2)
    w2i_t = w2i.rearrange("(v ko p) m -> p v ko m", p=P, v=2)
    xT_t = xTi.rearrange("(ko p) v b -> p ko v b", p=P)
    yT_t = yT.rearrange("(mo p) b -> p mo b", p=P)
    yT8_t = yT8.rearrange("(mo p) b -> p mo b", p=P) if y8_tiles else None

    # chunk boundaries for the startup split of w0 (by slot) / x(0) (by
    # k-group): the first matmuls need only low slots/k, so interleaving
    # chunks lets the PE start ~2us earlier than a monolithic load.
    kb = kb_list or [0, 4, 7]
    sb = sb_list or [0, 6, 8]
    in_split = len(kb) - 1
    if last_bounds is None:
        last_bounds = LAST_BOUNDS
    if y_pattern_last is None:
        y_pattern_last = y_pattern
    if y_pattern_14 is None:
        y_pattern_14 = y_pattern

    with tile.TileContext(nc) as tc:
        with (
            tc.tile_pool(name="wp", bufs=1) as wp,
            tc.tile_pool(name="xp", bufs=xp_bufs) as xp,
            tc.tile_pool(name="h1p", bufs=hp_bufs) as h1p,
            tc.tile_pool(name="h2p", bufs=hp_bufs) as h2p,
            tc.tile_pool(name="pp", bufs=(pp_bufs2 if pair_drain else pp_bufs),
                         space="PSUM") as pp,
            tc.tile_pool(name="pq", bufs=pq_bufs, space="PSUM") as pq,
            tc.tile_pool(name="op", bufs=op_bufs) as op,
        ):
            w0_sb = wp.tile([P, NB0, 2, H], F8)
            bia_sb = wp.tile([P, MO0 + MO1 + MO2], F32)
            b0_sb = bia_sb[:, 0:MO0] if bias_merge else None
            b1s_sb_m = bia_sb[:, MO0:MO0 + MO1]
            b2_sb_m = bia_sb[:, MO0 + MO1:]
            b0_sb = b0_sb if bias_merge else wp.tile([P, MO0], F32)
            w1_sb = wp.tile([P, KO1, H], F16)
            w2_sb = wp.tile([P, 2, KO2, S], F8)
            b1s_sb = wp.tile([P, MO1], F32)
            b2_sb = wp.tile([P, MO2], F32)

            x_tiles = {}
            h1_tiles = {}
            h2_tiles = {}

            # work items: (col0, width, last_use_of_x_tile). Tiles 0..NBT-2
            # are full 512-wide; the final tile is split into two 256-wide
            # halves so the drain->store->sem tail chain at the very end is
            # half as deep.
            if tail_split:
                items = [(t * BT, BT) for t in range(NBT - 1)]
                items += [((NBT - 1) * BT, BT // 2),
                          ((NBT - 1) * BT + BT // 2, BT // 2)]
            else:
                items = [(t * BT, BT) for t in range(NBT)]
            NIT = len(items)

            def load_x(t, split=1):
                # hi and lo planes load as separate DMAs: the interleaved
                # 4-dim access pattern cannot be balanced by the DMA engine.
                # xlo-skipped tiles only ship the hi plane (half the bytes).
                x_sb = xp.tile([P, KO0, 2, BT], F8, tag="x")
                bsl = slice(t * BT, (t + 1) * BT)
                chunks = ([(0, KO0)] if split == 1
                          else list(zip(kb[:-1], kb[1:])))
                lmax = (0 if t in xlo_skip else
                        4 if t in xhalf_skip else
                        6 if t in xq_skip else KO0)
                for lo, hi in chunks:
                    nc.sync.dma_start(x_sb[:, lo:hi, 0, :],
                                      xT_t[:, lo:hi, 0, bsl])
                    h2 = min(hi, lmax)
                    if h2 > lo:
                        nc.sync.dma_start(x_sb[:, lo:h2, 1, :],
                                          xT_t[:, lo:h2, 1, bsl])
                x_tiles[t] = x_sb

            def s0(i):
                # L0 fp8 DoubleRow matmuls + fused 1/64-scale + bias + ReLU
                # -> h1(i). 11 DR instructions per mj (vs 14 fp16 matmuls at
                # twice the per-row cost): slots 0..6 are the main+x-residual
                # pairs per k-group, slot 7 re-injects the odd group's
                # W-residual, slots 8..10 pair the W-residuals of adjacent
                # k-groups against their x_hi planes.
                col0, w = items[i]
                t, off = col0 // BT, col0 % BT
                x_sb = x_tiles[t]
                if off + w == BT:
                    x_tiles.pop(t)
                bsl = slice(off, off + w)
                h1 = h1p.tile([P, KO1, w], F16, tag="h1")
                skip_lo = t in xlo_skip
                half_lo = t in xhalf_skip
                q_lo = t in xq_skip
                ns = (7 if skip_lo else 9 if half_lo
                      else 10 if q_lo else NS0)
                for mj in range(MO0):
                    msl = slice(mj * P, (mj + 1) * P)
                    ps = pp.tile([P, w], F32, tag="ps")
                    for s in range(ns):
                        if skip_lo:
                            # 7-instr hi-only schedule: 3 {Whi64 pair}@xhi
                            # pairs, the fused {Whi64_6, Wlo64_6} @
                            # {xhi6, xhi6} block (all in weight chunk 1),
                            # then 3 {Wlo64 pair}@xhi pairs from chunk 2
                            if s <= 2:            # {Whi64_2j, Whi64_2j+1}
                                lhsT = w0_sb[:, s, :, msl]
                                rhs = x_sb[:, 2 * s:2 * s + 2, 0, bsl]
                            elif s == 3:          # fused k6 hi+lo
                                lhsT = w0_sb[:, 3, :, msl]
                                rhs = x_sb[:, KO0 - 1:KO0, 0,
                                           bsl].to_broadcast((P, 2, w))
                            else:                 # {Wlo64_2j, Wlo64_2j+1}
                                j = s - 4
                                lhsT = w0_sb[:, 4 + j, :, msl]
                                rhs = x_sb[:, 2 * j:2 * j + 2, 0, bsl]
                        elif q_lo:
                            # 10-instr quarter schedule: full hi+lo
                            # compensation for k0..5 via bcast-Whi, the
                            # fused {Whi64_6, Wlo64_6} block for k6 (its
                            # x_lo_6 product is the only one dropped), and
                            # 3 W-residual pairs
                            if s <= 5:            # Whi64_s @ {xhi_s, xlo_s}
                                lhsT = w0_sb[:, s // 2, s % 2:s % 2 + 1,
                                             msl].to_broadcast((P, 2, P))
                                rhs = x_sb[:, s, :, bsl]
                            elif s == 6:          # fused k6 hi+lo
                                lhsT = w0_sb[:, 3, :, msl]
                                rhs = x_sb[:, KO0 - 1:KO0, 0,
                                           bsl].to_broadcast((P, 2, w))
                            else:                 # {Wlo64_2j, Wlo64_2j+1}
                                j = s - 7
                                lhsT = w0_sb[:, 4 + j, :, msl]
                                rhs = x_sb[:, 2 * j:2 * j + 2, 0, bsl]
                        elif half_lo:
                            # 9-instr hybrid: x_lo compensation for k0..3
                            # only (bcast-Whi serving hi+lo), hi-pairs and
                            # the fused block for k4..6, 3 W-residual pairs
                            if s <= 3:            # Whi64_s @ {xhi_s, xlo_s}
                                lhsT = w0_sb[:, s // 2, s % 2:s % 2 + 1,
                                             msl].to_broadcast((P, 2, P))
                                rhs = x_sb[:, s, :, bsl]
                            elif s == 4:          # {Whi64_4, Whi64_5} @ xhi
                                lhsT = w0_sb[:, 2, :, msl]
                                rhs = x_sb[:, 4:6, 0, bsl]
                            elif s == 5:          # fused k6 hi+lo
                                lhsT = w0_sb[:, 3, :, msl]
                                rhs = x_sb[:, KO0 - 1:KO0, 0,
                                           bsl].to_broadcast((P, 2, w))
                            else:                 # {Wlo64_2j, Wlo64_2j+1}
                                j = s - 6
                                lhsT = w0_sb[:, 4 + j, :, msl]
                                rhs = x_sb[:, 2 * j:2 * j + 2, 0, bsl]
                        elif s <= KO0 - 1:
                            # Whi64_k broadcast to both halves: x_lo ships
                            # UNSCALED, so main and x-residual products share
                            # one 64x weight plane
                            lhsT = w0_sb[:, s // 2, s % 2:s % 2 + 1,
                                         msl].to_broadcast((P, 2, P))
                            rhs = x_sb[:, s, :, bsl]
                        elif s <= KO0 + 2:        # {Wlo64_2j, Wlo64_2j+1} @ xhi
                            j = s - KO0
                            lhsT = w0_sb[:, 4 + j, :, msl]
                            rhs = x_sb[:, 2 * j:2 * j + 2, 0, bsl]
                        else:                     # {Wlo64_6, zero}
                            lhsT = w0_sb[:, 7, :, msl]
                            rhs = x_sb[:, KO0 - 1, :, bsl]
                        nc.tensor.matmul(ps[:], lhsT, rhs,
                                         start=(s == 0), stop=(s == ns - 1),
                                         perf_mode=DR)
                    b0v = b0_sb[:, mj:mj + 1]
                    if h1_pattern[t] == "A":
                        nc.scalar.activation(h1[:, mj, :], ps[:], RELU,
                                             bias=b0v, scale=1.0)
                    else:
                        nc.vector.tensor_scalar(h1[:, mj, :], ps[:],
                                                b0v, 0.0,
                                                mybir.AluOpType.add,
                                                mybir.AluOpType.max)
                h1_tiles[i] = h1

            def s1(i):
                # L1 matmuls (fp16), then drains build the 32x-scaled fp8
                # h2 planes L2's DoubleRow needs:
                #   a32 = fp16(relu(32*z + 32*b1))      (ScalarE)
                #   b32 = fp8(a32)        = 32*h2_hi    (DVE copy)
                #   c   = fp8(a32 - b32)  = 32*h2_lo    (DVE sub)
                #   h4  = fp8(a32/8)      =  4*h2_hi    (DVE scale)
                _, w = items[i]
                h1 = h1_tiles.pop(i)
                a32 = h2p.tile([P, KO2, w], F16, tag="a32")
                b32 = h2p.tile([P, KO2, w], F8, tag="b32")
                cc = h2p.tile([P, KO2, w], F8, tag="cc")
                h4 = h2p.tile([P, KO2, w], F8, tag="h4")
                for mj in range(MO1):
                    ps = pp.tile([P, w], F32, tag="ps")
                    for k in range(KO1):
                        nc.tensor.matmul(ps[:], w1_sb[:, k, mj * P:(mj + 1) * P],
                                         h1[:, k, :],
                                         start=(k == 0), stop=(k == KO1 - 1))
                    # w1 ships half-scaled (w1/2 against 64x h1), so psum is
                    # already 32*z1 and the drain is relu(psum + 32*b1) --
                    # expressible on either Act or DVE
                    bsrc = (b1s_sb_m if bias_merge else b1s_sb)[:, mj:mj + 1]
                    if a32_pattern[i] == "A":
                        nc.scalar.activation(a32[:, mj, :], ps[:], RELU,
                                             bias=bsrc, scale=1.0)
                    else:
                        nc.vector.tensor_scalar(a32[:, mj, :], ps[:],
                                                bsrc, 0.0,
                                                mybir.AluOpType.add,
                                                mybir.AluOpType.max)
                    nc.vector.tensor_scalar(b32[:, mj, :], a32[:, mj, :],
                                            1.0, None, mybir.AluOpType.mult)
                    if i not in cc_skip:
                        # cc_skip tiles drop the h2-residual product (costs
                        # 5e-3 of the 2e-2 error budget): its DVE production
                        # chain is exactly what the last L2 matmuls stall on
                        cc_eng = (nc.gpsimd if (cc_pool or i in cc_pool_tiles)
                                  else nc.vector)
                        cc_eng.tensor_tensor(cc[:, mj, :], a32[:, mj, :],
                                             b32[:, mj, :],
                                             mybir.AluOpType.subtract)
                    nc.gpsimd.tensor_scalar(h4[:, mj, :], a32[:, mj, :],
                                            0.125, None, mybir.AluOpType.mult)
                h2_tiles[i] = (b32, cc, h4)

            def s2(i):
                # L2 fp8 DoubleRow matmuls: 3 DR instructions per mj
                # (W2hi@b32 + W2hi@cc + W2lo8@h4 = 32*(W2@h2)), then a
                # 1/32-scale + bias drain spread over ScalarE/DVE/Pool ->
                # y(i), stored in chunks so the drain overlaps the DMA.
                col0, w = items[i]
                b32, cc, h4 = h2_tiles.pop(i)
                bsl = slice(col0, col0 + w)
                y8 = i in y8_tiles
                yd_t = yT8_t if y8 else yT_t
                o_sb = op.tile([P, MO2, w], F8 if y8 else F16, tag="o")
                if i == NIT - 1:
                    # finer chunks at the very end: the last store is on the
                    # critical path (drain -> DMA -> sem), so keep it small
                    bounds = last_bounds
                else:
                    bounds = [MO2 * (g + 1) // y_split for g in range(y_split)]
                if i >= NIT - 1:
                    pat = y_pattern_last
                elif i == NIT - 2:
                    pat = y_pattern_14
                else:
                    pat = y_pattern
                if y_pat_map:
                    pat = y_pat_map.get(i, y_pat_map.get(str(i), pat))
                split = i >= NIT - last_split
                last_eng = nc.sync

                if pair_drain:
                    # drain mj pairs from a 2-bank PSUM tile in one op; b2 is
                    # added on the host (final layer is linear), so no bias
                    # operand is needed and the pair op is legal on Act
                    skip_cc = i in cc_skip
                    for mp in range(MO2 // 2):
                        ps2 = pq.tile([P, 2, w], F32, tag="ps2")
                        for half in range(2):
                            mj = 2 * mp + half
                            msl = slice(mj * P, (mj + 1) * P)
                            nc.tensor.matmul(ps2[:, half, :],
                                             w2_sb[:, 0, :, msl], b32[:],
                                             start=True, stop=False,
                                             perf_mode=DR)
                            if not skip_cc:
                                nc.tensor.matmul(ps2[:, half, :],
                                                 w2_sb[:, 0, :, msl], cc[:],
                                                 start=False, stop=False,
                                                 perf_mode=DR)
                            nc.tensor.matmul(ps2[:, half, :],
                                             w2_sb[:, 1, :, msl], h4[:],
                                             start=False, stop=True,
                                             perf_mode=DR)
                        if pat[2 * mp] == "A":
                            nc.scalar.activation(o_sb[:, 2 * mp:2 * mp + 2, :],
                                                 ps2[:], IDENT,
                                                 scale=1.0 / 32.0)
                        else:
                            nc.vector.tensor_scalar(o_sb[:, 2 * mp:2 * mp + 2, :],
                                                    ps2[:], 1.0 / 32.0, None,
                                                    mybir.AluOpType.mult)
                        nc.sync.dma_start(yT_t[:, 2 * mp:2 * mp + 2, bsl],
                                          o_sb[:, 2 * mp:2 * mp + 2, :])
                    return

                def drain(dst, src, bias, eng):
                    # GPSIMD cannot read PSUM: drains go Act / DVE only
                    if eng == "A":
                        nc.scalar.activation(dst, src, IDENT,
                                             bias=bias, scale=1.0 / 32.0)
                        return nc.scalar
                    nc.vector.tensor_scalar(dst, src, 1.0 / 32.0, bias,
                                            mybir.AluOpType.mult,
                                            mybir.AluOpType.add)
                    return nc.vector

                deferred = []
                for mj in range(MO2):
                    msl = slice(mj * P, (mj + 1) * P)
                    ps = pp.tile([P, w], F32, tag="ps")
                    prods = {"b": (0, b32), "c": (1, cc), "h": (2, h4)}
                    seq = [prods[ch] for ch in prod_order
                           if not (ch == "c" and i in cc_skip)]
                    for n, (kind, rhs) in enumerate(seq):
                        lhsT = w2_sb[:, 1 if kind == 2 else 0, :, msl]
                        nc.tensor.matmul(ps[:], lhsT, rhs[:],
                                         start=(n == 0),
                                         stop=(n == len(seq) - 1),
                                         perf_mode=DR)
                    bias = (b2_sb_m if bias_merge else b2_sb)[:, mj:mj + 1]
                    if split:
                        h = w // 2
                        e0 = "A" if pat[mj] == "A" else "D"
                        e1 = "D" if e0 == "A" else "A"
                        drain(o_sb[:, mj, :h], ps[:, :h], bias, e0)
                        last_eng = drain(o_sb[:, mj, h:], ps[:, h:], bias, e1)
                    else:
                        last_eng = drain(o_sb[:, mj, :], ps[:], bias, pat[mj])
                    if (mj + 1) in bounds:
                        gi = bounds.index(mj + 1)
                        lo = 0 if gi == 0 else bounds[gi - 1]
                        msl = slice(lo, mj + 1)
                        if tail_par and i == NIT - 1:
                            deferred.append((gi, msl))
                            continue
                        if i in store_alt:
                            se = nc.scalar
                        elif store_eng == "sp":
                            se = nc.sync
                        elif store_eng == "act":
                            se = nc.scalar
                        elif store_eng == "prod":
                            # only SP/Act can drive HWDGE; DVE-drained chunks
                            # fall back to SP
                            se = last_eng if last_eng is nc.scalar else nc.sync
                        else:  # rotate sp/act by chunk index
                            se = (nc.sync, nc.scalar)[gi % 2]
                        se.dma_start(yd_t[:, msl, bsl], o_sb[:, msl, :])
                for gi, msl in deferred:
                    se = (nc.sync, nc.scalar)[gi % 2]
                    se.dma_start(yd_t[:, msl, bsl], o_sb[:, msl, :])

            # -- PE warm-up: the cost model ramps the PE clock (0.65 GHz ->
            # 1.2 GHz -> 2.4 GHz over ~3us of continuous execution). Run tiny
            # matmuls on memset data while the first DMAs are in flight so
            # the ramp finishes before the real matmuls start.
            if wu_n:
                wu = wp.tile([P, P + wu_ap], F16, tag="wu")
                (nc.gpsimd if wu_pool else nc.vector).memset(wu[:], 0.0)
                wu_ps = pp.tile([P, BT], F32, tag="ps")
                for _ in range(wu_n):
                    nc.tensor.matmul(wu_ps[:, :wu_ap], wu[:, :P],
                                     wu[:, P:P + wu_ap], start=True, stop=True)

            # -- prologue: interleave w0/x0 k-chunks so the PE starts early;
            # x(1)/x(2) go ahead of the weights the PE won't need for a while
            # (every DMA serializes on the HWDGE + DMA-engine devices).
            x0_sb = xp.tile([P, KO0, 2, BT], F8, tag="x")
            xe = nc.scalar if x_eng_alt else nc.sync
            nv0 = 1 if 0 in xlo_skip else 2
            for g in range(in_split):
                ss = slice(sb[g], sb[g + 1])
                ks = slice(kb[g], kb[g + 1])
                if g > 0 and g1_swap:
                    for v in range(nv0):
                        xe.dma_start(x0_sb[:, ks, v, :], xT_t[:, ks, v, 0:BT])
                if sb[g + 1] > sb[g]:
                    nc.sync.dma_start(w0_sb[:, ss, :, :], w0a_t[:, ss, :, :])
                if g == 0 or not g1_swap:
                    for v in range(nv0):
                        xe.dma_start(x0_sb[:, ks, v, :], xT_t[:, ks, v, 0:BT])
                if g == 0:
                    # the three tiny bias vectors gate the whole first
                    # vertical chain (b0 unblocks Act's SEQ + the one-time
                    # act-table load; b1s/b2 gate a32(0)/y(0)) -- ship them
                    # right behind the first w0/x0 chunks
                    if bias_merge:
                        nc.sync.dma_start(bia_sb[:],
                                          bia.rearrange("(mo p) -> p mo",
                                                        p=P))
                    else:
                        nc.sync.dma_start(b0_sb[:],
                                          b0.rearrange("(mo p) -> p mo",
                                                       p=P))
                        nc.sync.dma_start(b1s_sb[:],
                                          b1s.rearrange("(mo p) -> p mo",
                                                        p=P))
                        nc.sync.dma_start(b2_sb[:],
                                          b2.rearrange("(mo p) -> p mo",
                                                       p=P))
            x_tiles[0] = x0_sb
            load_x(1, split=2)
            nc.sync.dma_start(w1_sb[:], w1t.rearrange("(ko p) m -> p ko m", p=P))
            load_x(2, split=2)

            def load_w2(vs=(0, 1)):
                if w2_msplit:
                    hm = S // 2
                    for v in range(2):
                        nc.sync.dma_start(w2_sb[:, v, :, :hm],
                                          w2i_t[:, v, :, :hm])
                    for v in range(2):
                        nc.sync.dma_start(w2_sb[:, v, :, hm:],
                                          w2i_t[:, v, :, hm:])
                else:
                    for v in vs:
                        nc.sync.dma_start(w2_sb[:, v, :, :], w2i_t[:, v, :, :])
                if 1 not in vs:
                    return

            if w2v_split:
                load_w2(vs=(0,))
            elif w2_after is None:
                load_w2()

            # -- software-pipelined main loop --
            loaded = {0, 1, 2}
            for i in range(NIT + 2):
                for t_pre in range(3, min(ahead + i, NIT)):
                    if i == 0 and t_pre not in loaded:
                        loaded.add(t_pre)
                        load_x(t_pre)
                if i + ahead <= NIT - 1:
                    t_need = items[i + ahead][0] // BT
                    if t_need not in loaded:
                        loaded.add(t_need)
                        load_x(t_need)
                if w2_after is not None and i == w2_after:
                    load_w2()
                if w2v_split and i == 0:
                    load_w2(vs=(1,))
                if i == 3 and i3_special:
                    # x(3) arrives bytes-bound ~1us into this iteration;
                    # running the older tiles' stages first hides the wait
                    s1(i - 1); s2(i - 2); s0(i)
                    continue
                stages = {"0": (s0, i), "1": (s1, i - 1), "2": (s2, i - 2)}
                order = stage_order
                if winddown_21 and i > NIT - 1:
                    order = "".join(reversed(stage_order))
                for ch in order:
                    fn, j = stages[ch]
                    if 0 <= j <= NIT - 1:
                        fn(j)
    nc.compile()
    return nc


def kernel(state, action, W0, b0, W1, b1, W2, b2,
           D0, U0, D1, U1, D2, U2):
    global LAST_RESULT
    state = np.asarray(state, dtype=np.float32)
    action = np.asarray(action, dtype=np.float32)

    def fold(W, U, D):
        # exact LoRA merge, done in float64 to keep the fold itself lossless
        We = W.astype(np.float64) + LORA_SCALE * (
            U.astype(np.float64) @ D.astype(np.float64))
        return We.T  # [in, out], float64

    w0f = fold(np.asarray(W0), np.asarray(U0), np.asarray(D0))
    # w1 absorbs the 1/64 L0-drain scale (h1 is stored 64x-scaled so its
    # relu drain needs no scale operand and can run on DVE as add+max)
    # w1 absorbs the 1/64 L0-drain scale AND the 32x a32 pre-scale
    # (net /2): both drains become scale-free add(+bias)/max ops that can
    # run on ScalarE or DVE
    w1t = np.ascontiguousarray(
        (fold(np.asarray(W1), np.asarray(U1), np.asarray(D1))
         * (32.0 / LO_SCALE)).astype(np.float16))
    # L2 fp8 hi/lo planes: [W2hi; W2lo8] with W2lo8 = fp8(8*(W2 - W2hi))
    w2f = fold(np.asarray(W2), np.asarray(U2), np.asarray(D2)).astype(np.float32)
    w2hi = w2f.astype(F8NP)
    w2lo8 = ((w2f - w2hi.astype(np.float32)) * 8.0).astype(F8NP)
    w2i = np.ascontiguousarray(np.concatenate([w2hi, w2lo8], axis=0))

    # fp8 hi/lo decomposition of W0 for the DoubleRow blocks (see _build).
    # x_lo ships unscaled, so one 64*Whi plane serves both DR halves
    # (broadcast lhsT); blocks: 4x{Whi64 pair}, 3x{Wlo64 cc pair},
    # {Wlo64_6, zero}. All scale shifts are exact powers of two.
    w0f = w0f.astype(np.float32)                     # [F0, H]
    whi = w0f.astype(F8NP).astype(np.float32)        # fp8(W)
    whi64 = (whi * LO_SCALE).astype(F8NP)
    wlo64 = ((w0f - whi) * LO_SCALE).astype(F8NP)
    w0a = np.zeros((NB0, P, 2, H), dtype=F8NP)
    for k in range(KO0):
        w0a[k // 2, :, k % 2] = whi64[k * P:(k + 1) * P]
    for j in range(3):
        w0a[4 + j, :, 0] = wlo64[2 * j * P:(2 * j + 1) * P]
        w0a[4 + j, :, 1] = wlo64[(2 * j + 1) * P:(2 * j + 2) * P]
    w0a[7, :, 0] = wlo64[6 * P:]
    # slot 3 v1 was a dead zero plane: fuse Wlo64_6 there so the hi-only
    # schedule's k6 block ({Whi64_6, Wlo64_6} @ {xhi6, xhi6}) lives in the
    # first weight chunk. Normal tiles broadcast slot 3 v0 only.
    w0a[3, :, 1] = wlo64[6 * P:]
    w0a = np.ascontiguousarray(w0a.reshape(NB0 * P, 2 * H))
    b0 = np.ascontiguousarray(np.asarray(b0, dtype=np.float32) * LO_SCALE)
    b1s = np.ascontiguousarray(np.asarray(b1, dtype=np.float32) * 32.0)
    b2 = np.ascontiguousarray(np.asarray(b2, dtype=np.float32))

    # feature-major input as an fp8 hi/lo pair, sharded over cores on batch
    xT = np.empty((F0, B), dtype=np.float32)
    xT[:S] = state.T
    xT[S:] = action.T
    x_hi = xT.astype(F8NP)
    x_lo = (xT - x_hi.astype(np.float32)).astype(F8NP)  # unscaled residual
    xTi = np.empty((F0, 2, B), dtype=F8NP)
    xTi[:, 0, :] = x_hi
    xTi[:, 1, :] = x_lo

    if not _NC_CACHE:
        _NC_CACHE.append(_build())
    nc = _NC_CACHE[0]

    bia = np.ascontiguousarray(np.concatenate([b0, b1s, b2]))
    in_maps = [
        {
            "xTi": np.ascontiguousarray(xTi[:, :, c * BC:(c + 1) * BC]),
            "w0a": w0a, "w1t": w1t, "w2i": w2i,
            "b0": b0, "b1s": b1s, "b2": b2, "bia": bia,
        }
        for c in range(NCORES)
    ]
    res = bass_utils.run_bass_kernel_spmd(nc, in_maps,
                                          core_ids=list(range(NCORES)))
    LAST_RESULT = res

    out = np.empty((B, S), dtype=np.float32)
    for c in range(NCORES):
        out[c * BC:(c + 1) * BC, :] = res.results[c]["yT"].T.astype(np.float32)
    return out

